# revision 2
# baseline (speedup 1.0000x reference)
"""Trainium2 Bass kernel for nn_GatedFusionModel (2-layer GAT + gated fusion
+ pair predictor), SPMD over 8 NeuronCores.

Sharding: nodes 2500/core (dst-sorted edge blocks of 128 dst), weights
replicated, pairs sorted by owner(u). Three AllGathers exchange the per-layer
node tables; edge softmax-aggregation runs as one-hot S_T matmuls with a
ones-column producing softmax denominators (max-subtraction provably
unnecessary for this model's value ranges).
"""
import sys
sys.path.insert(0, '/opt/trn_rl_repo')
import numpy as np
import ml_dtypes

from concourse import bacc, tile, mybir
from concourse.bass_utils import run_bass_kernel_spmd

BF16 = ml_dtypes.bfloat16
FP32 = np.float32

N, E, P = 20000, 320000, 100000
TC, HID, H, OUT, PH = 256, 64, 4, 256, 512
NCORES = 8
NSH = N // NCORES            # 2500
NSH_PAD = 2560
DBLK = 128
NBLK = NSH_PAD // DBLK       # 20
TB = 18                      # edge tiles per block (max block degree <= 2304)
EB = TB * 128                # 2304 edge slots per block
PBATCH = 512
SLOT1, NRHS1 = HID + 1, H * (HID + 1)   # 65, 260
NRHS2 = OUT + 1                          # 257
ROW = 384                                # table row cols (bf16) = 768B

dt = mybir.dt
B_LEVEL = 5


def wrap_idx(idx):
    """[n] int -> [128, n//16] int16 in the 16-partition wrapped+replicated layout."""
    n = len(idx)
    assert n % 16 == 0
    w = np.asarray(idx, np.int16).reshape(n // 16, 16).T  # [16, n/16]
    return np.tile(w, (8, 1))  # [128, n/16]


def preprocess(edge_index, u_nodes, v_nodes):
    src = np.concatenate([edge_index[0], np.arange(N)]).astype(np.int64)
    dst = np.concatenate([edge_index[1], np.arange(N)]).astype(np.int64)
    order = np.argsort(dst, kind="stable")
    src_s, dst_s = src[order], dst[order]
    blk_counts = np.bincount(dst_s // DBLK, minlength=N // DBLK)
    assert blk_counts.max() <= EB, blk_counts.max()

    owner = u_nodes // NSH
    porder = np.argsort(owner, kind="stable")
    pmax = int(np.bincount(owner, minlength=NCORES).max())
    pmax = ((pmax + PBATCH - 1) // PBATCH) * PBATCH
    nbat = pmax // PBATCH

    cores = []
    for k in range(NCORES):
        lo, hi = k * NSH, (k + 1) * NSH
        m = (dst_s >= lo) & (dst_s < hi)
        es, ed = src_s[m], dst_s[m] - lo
        esrc = np.zeros((NBLK, 128, EB // 16), np.int16)
        edst = np.zeros((NBLK, 128, EB // 16), np.int16)
        erel = np.zeros((NBLK, 128, TB), BF16)
        for b in range(NBLK):
            bm = (ed >= b * DBLK) & (ed < (b + 1) * DBLK)
            cnt = int(bm.sum())
            sp = np.zeros(EB, np.int64); sp[:cnt] = es[bm]
            rp = np.full(EB, 128, np.int64); rp[:cnt] = ed[bm] - b * DBLK
            dg = np.zeros(EB, np.int64)
            dg[:cnt] = lo + b * DBLK + (ed[bm] - b * DBLK)
            esrc[b] = wrap_idx(sp)
            edst[b] = wrap_idx(dg)
            # gathered edge i lands at [partition i%128, tile i//128]
            erel[b] = rp.reshape(TB, 128).T.astype(BF16)
        pm = porder[owner[porder] == k]
        npad = pmax - len(pm)
        u_loc = np.concatenate([u_nodes[pm] - lo, np.zeros(npad, np.int64)])
        v_glob = np.concatenate([v_nodes[pm], np.zeros(npad, np.int64)])
        uidx = np.stack([wrap_idx(u_loc[j * PBATCH:(j + 1) * PBATCH]) for j in range(nbat)])
        vidx = np.stack([wrap_idx(v_glob[j * PBATCH:(j + 1) * PBATCH]) for j in range(nbat)])
        cores.append(dict(esrc=esrc, edst=edst, erel=erel, uidx=uidx, vidx=vidx,
                          pidx=pm, np_real=len(pm)))
    return cores, pmax, nbat


def host_weights(inp):
    f = lambda k: np.asarray(inp[k], FP32)
    W1, a1s, a1d, b1 = f("W1"), f("a1s"), f("a1d"), f("b1")
    W2, a2s, a2d, b2 = f("W2"), f("a2s"), f("a2d"), f("b2")
    gw1, gb1, gw2, gb2 = f("gw1"), f("gb1"), f("gw2"), f("gb2")
    pw1, pb1, pw2, pb2 = f("pw1"), f("pb1"), f("pw2"), f("pb2")

    A1S = np.stack([W1[:, h * HID:(h + 1) * HID] @ a1s[h] for h in range(H)], 1)
    A1D = np.stack([W1[:, h * HID:(h + 1) * HID] @ a1d[h] for h in range(H)], 1)
    W1ext = np.concatenate([W1, A1S, A1D], 1)                       # [256,264]
    W2ext = np.concatenate([W2, (W2 @ a2s[0])[:, None], (W2 @ a2d[0])[:, None]], 1)
    bc = lambda v: np.broadcast_to(v[None, :], (128, len(v))).copy()
    w = dict(
        W1e=W1ext.reshape(2, 128, 264).transpose(1, 0, 2).astype(BF16),
        W2e=W2ext.reshape(2, 128, 258).transpose(1, 0, 2).astype(BF16),
        GW1=gw1.reshape(4, 128, 256).transpose(1, 0, 2).astype(BF16),
        PWT=pw1[:OUT].reshape(2, 128, PH).transpose(1, 0, 2).astype(BF16),
        PWB=pw1[OUT:].reshape(2, 128, PH).transpose(1, 0, 2).astype(BF16),
        PW2T=pw2[:, 0].reshape(4, 128).T.astype(BF16).copy(),       # [128,4]
        B1=bc(b1).astype(FP32), B2=bc(b2).astype(FP32),
        GB1=bc(gb1).astype(FP32),
        GW2=bc(gw2[:, 0]).astype(BF16),
        PB1=bc(pb1).astype(FP32),                                   # [128,512]
        GB2=np.full((128, 1), gb2[0], FP32),
        PB2=np.full((128, 1), pb2[0], FP32),
        IOTA=np.broadcast_to(np.arange(128, dtype=FP32), (128, 128)).astype(BF16).copy(),
        IDENT=np.eye(128, dtype=FP32).astype(BF16),
    )
    return w


WSPEC = dict(
    W1e=([128, 2, 264], dt.bfloat16), W2e=([128, 2, 258], dt.bfloat16),
    GW1=([128, 4, 256], dt.bfloat16),
    PWT=([128, 2, PH], dt.bfloat16), PWB=([128, 2, PH], dt.bfloat16),
    PW2T=([128, 4], dt.bfloat16),
    B1=([128, 256], dt.float32), B2=([128, 256], dt.float32),
    GB1=([128, 256], dt.float32), GW2=([128, 256], dt.bfloat16),
    PB1=([128, PH], dt.float32),
    GB2=([128, 1], dt.float32), PB2=([128, 1], dt.float32),
    IOTA=([128, 128], dt.bfloat16), IDENT=([128, 128], dt.bfloat16),
)


def build(nbat, pmax, stop_after='full', bench=None):
    # bench=("loop", R): wrap compute phases in For_i(0,R), skip collectives.
    # bench=("coll", K): only K x 3 AllGathers.
    nc = bacc.Bacc("TRN2", target_bir_lowering=False, debug=False, num_devices=NCORES)
    core_ids = list(range(NCORES))

    # ---------------- inputs ----------------
    xT_ext = nc.declare_dram_parameter("xT", [128, 2, NSH_PAD], dt.bfloat16, isOutput=False)
    x_ext = nc.declare_dram_parameter("x", [NSH_PAD, 256], dt.float32, isOutput=False)
    esrc_ext = nc.declare_dram_parameter("esrc", [NBLK, 128, EB // 16], dt.int16, isOutput=False)
    edst_ext = nc.declare_dram_parameter("edst", [NBLK, 128, EB // 16], dt.int16, isOutput=False)
    erel_ext = nc.declare_dram_parameter("erel", [NBLK, 128, TB], dt.bfloat16, isOutput=False)
    uidx_ext = nc.declare_dram_parameter("uidx", [nbat, 128, PBATCH // 16], dt.int16, isOutput=False)
    vidx_ext = nc.declare_dram_parameter("vidx", [nbat, 128, PBATCH // 16], dt.int16, isOutput=False)
    wext = {k: nc.declare_dram_parameter(k, shp, d, isOutput=False) for k, (shp, d) in WSPEC.items()}
    out_ext = nc.declare_dram_parameter("pout", [nbat, PBATCH], dt.float32, isOutput=True)

    import contextlib
    benchmode = bench[0] if bench else None
    with tile.TileContext(nc) as tc:
        cpool = tc.alloc_tile_pool(name="consts", bufs=1)
        dram = tc.alloc_tile_pool(name="dram", bufs=1, space="DRAM")

        tab1_loc = dram.tile([NSH, ROW], dt.bfloat16)
        tab1_ag = dram.tile([NCORES, NSH, ROW], dt.bfloat16, addr_space="Shared")
        tab2_loc = dram.tile([NSH, ROW], dt.bfloat16)
        tab2_ag = dram.tile([NCORES, NSH, ROW], dt.bfloat16, addr_space="Shared")
        hf_loc = dram.tile([NSH, OUT], dt.bfloat16)
        hf_ag = dram.tile([NCORES, NSH, OUT], dt.bfloat16, addr_space="Shared")
        a_loc = dram.tile([NSH_PAD, PH], dt.bfloat16)

        W = {}
        for k, (shp, d) in WSPEC.items():
            W[k] = cpool.tile(shp, d, tag=f"w_{k}", name=f"w_{k}")
            nc.sync.dma_start(out=W[k][:], in_=wext[k][:])
        xT = cpool.tile([128, 2, NSH_PAD], dt.bfloat16, tag="xT")
        nc.sync.dma_start(out=xT[:], in_=xT_ext[:])

        _benchstack = contextlib.ExitStack()
        if benchmode == "loop":
            _benchstack.enter_context(tc.For_i(0, bench[1], 1))

        # ---------------- phase A: node linear L1 -> table1 ----------------
        with tc.tile_pool(name="pa", bufs=3) as pool, \
             tc.tile_pool(name="psA", bufs=2, space="PSUM") as psA:
            for i in range(NBLK):
                ps = psA.tile([128, 264], dt.float32, tag="psA")
                for c in range(2):
                    nc.tensor.matmul(ps[:], xT[:, c, i * 128:(i + 1) * 128], W["W1e"][:, c, :],
                                     start=(c == 0), stop=(c == 1))
                tab = pool.tile([128, ROW], dt.bfloat16, tag="tabA")
                t4 = tab[:, 0:NRHS1].rearrange("p (h s) -> p h s", h=H)
                nc.scalar.copy(t4[:, :, 0:HID], ps[:, 0:256].rearrange("p (h c) -> p h c", h=H))
                nc.vector.memset(t4[:, :, HID:SLOT1], 1.0)
                nc.scalar.copy(tab[:, NRHS1:NRHS1 + 8], ps[:, 256:264])
                nc.vector.memset(tab[:, NRHS1 + 8:ROW], 0.0)
                lo = i * 128
                nr = min(128, NSH - lo)
                if nr > 0:
                    nc.sync.dma_start(out=tab1_loc[lo:lo + nr], in_=tab[0:nr])
        if stop_after not in ('A',) and benchmode is None:
            nc.gpsimd.collective_compute(
                "AllGather", mybir.AluOpType.bypass,
                ins=[tab1_loc.opt()], outs=[tab1_ag.opt()], replica_groups=[core_ids])

        # ---------------- edge phase (shared for L1/L2) ----------------
        def edge_phase(tab_ag, heads, slot, nrhs, out_cb):
            """out_cb(b, h_sbuf_f32[128, heads*(slot-1)]) consumes each block's result."""
            hid2 = slot - 1
            tabv = tab_ag[:, :, :].flatten_outer_dims()  # [20000, ROW]
            with tc.tile_pool(name="pe", bufs=2) as pool, \
                 tc.tile_pool(name="psE", bufs=2, space="PSUM") as psE:
                for b in range(NBLK):
                    g1 = pool.tile([128, TB, ROW], dt.bfloat16, tag="g1")
                    for q in range(3):
                        nc.gpsimd.dma_gather(
                            g1[:, 6 * q:6 * (q + 1), :], tabv,
                            idx_sb[b][:, 48 * q:48 * (q + 1)], 768, 768, ROW, elem_step=ROW)
                    if B_LEVEL == 1:
                        nc.sync.dma_start(out=tab2_loc[0:128], in_=g1[:, 0, :])
                        continue
                    g2 = pool.tile([128, TB, 128], dt.bfloat16, tag="g2")
                    for q in range(3):
                        nc.gpsimd.dma_gather(
                            g2[:, 6 * q:6 * (q + 1), :], tabv[:, 256:384],
                            idx_db[b][:, 48 * q:48 * (q + 1)], 768, 768, 128, elem_step=ROW)
                    rel = pool.tile([128, TB], dt.bfloat16, tag="rel")
                    nc.sync.dma_start(out=rel[:], in_=erel_ext[b])
                    st = pool.tile([128, TB, 128], dt.bfloat16, tag="st")
                    nc.vector.tensor_tensor(
                        out=st[:], in0=rel[:].unsqueeze(-1).broadcast_to([128, TB, 128]),
                        in1=W["IOTA"][:].unsqueeze(1).broadcast_to([128, TB, 128]),
                        op=mybir.AluOpType.is_equal)
                    if B_LEVEL == 2:
                        nc.sync.dma_start(out=tab2_loc[0:128, 0:TB*128].rearrange("p (t c) -> p t c", t=TB), in_=st[:])
                        continue
                    ev = pool.tile([128, TB, heads], dt.float32, tag="ev")
                    nc.vector.tensor_tensor(
                        out=ev[:], in0=g1[:, :, nrhs:nrhs + heads],
                        in1=g2[:, :, 8:8 + heads] if heads == 4 else g2[:, :, 1 + heads:1 + 2 * heads],
                        op=mybir.AluOpType.add)
                    lr1 = pool.tile([128, TB, heads], dt.float32, tag="lr1")
                    nc.vector.tensor_scalar(lr1[:], ev[:], 0.0, 0.8,
                                            mybir.AluOpType.max, mybir.AluOpType.mult)
                    lr2 = pool.tile([128, TB, heads], dt.float32, tag="lr2")
                    nc.vector.tensor_scalar(lr2[:], ev[:], 0.2, None, mybir.AluOpType.mult)
                    pt = pool.tile([128, TB, heads], dt.float32, tag="pt")
                    nc.vector.tensor_tensor(out=pt[:], in0=lr1[:], in1=lr2[:], op=mybir.AluOpType.add)
                    nc.scalar.activation(pt[:], pt[:], mybir.ActivationFunctionType.Exp)
                    rt = pool.tile([128, TB, nrhs], dt.bfloat16, tag="rt")
                    nc.vector.tensor_tensor(
                        out=rt[:].rearrange("p t (h s) -> p t h s", h=heads),
                        in0=g1[:, :, 0:nrhs].rearrange("p t (h s) -> p t h s", h=heads),
                        in1=pt[:].unsqueeze(-1).broadcast_to([128, TB, heads, slot]),
                        op=mybir.AluOpType.mult)
                    if B_LEVEL == 3:
                        nc.sync.dma_start(out=tab2_loc[0:128, 0:nrhs], in_=rt[:, 0, :])
                        continue
                    ps = psE.tile([128, nrhs], dt.float32, tag="psE")
                    for t in range(TB):
                        nc.tensor.matmul(ps[:], st[:, t, :], rt[:, t, :],
                                         start=(t == 0), stop=(t == TB - 1))
                    # epilogue: divide by denominator
                    den = pool.tile([128, heads], dt.float32, tag="den")
                    ps3 = ps[:].rearrange("p (h s) -> p h s", h=heads)
                    nc.vector.tensor_scalar_max(den[:], ps3[:, :, hid2], 1e-30)
                    rec = pool.tile([128, heads], dt.float32, tag="rec")
                    nc.vector.reciprocal(rec[:], den[:])
                    hsb = pool.tile([128, heads * hid2], dt.float32, tag="hsb")
                    nc.vector.tensor_tensor(
                        out=hsb[:].rearrange("p (h c) -> p h c", h=heads),
                        in0=ps3[:, :, 0:hid2],
                        in1=rec[:].unsqueeze(-1).broadcast_to([128, heads, hid2]),
                        op=mybir.AluOpType.mult)
                    if B_LEVEL == 4:
                        nc.sync.dma_start(out=tab2_loc[0:128, 0:heads*hid2], in_=hsb[:])
                        continue
                    out_cb(b, pool, hsb)

            # idx tiles loaded up-front (SBUF resident, small)
        idx_sb, idx_db = [], []

        def load_edge_idxs():
            for b in range(NBLK):
                ts_ = cpool.tile([128, EB // 16], dt.int16, tag=f"esrc{b}")
                nc.sync.dma_start(out=ts_[:], in_=esrc_ext[b])
                idx_sb.append(ts_)
                td_ = cpool.tile([128, EB // 16], dt.int16, tag=f"edst{b}")
                nc.sync.dma_start(out=td_[:], in_=edst_ext[b])
                idx_db.append(td_)
        if stop_after not in ('A', 'AG1'):
            load_edge_idxs()

        # ---------------- phase B: edge L1 + node L2 -> table2 ----------------
        with tc.tile_pool(name="pb2", bufs=2) as poolB, \
             tc.tile_pool(name="psB", bufs=2, space="PSUM") as psB:

            def consume_l1(b, pool, hsb):
                # hsb [128, 256] f32 = agg; h_elu = relu(h)+exp(min(h,0))-1, h = hsb+b1
                h = poolB.tile([128, 256], dt.float32, tag="hb")
                nc.vector.tensor_tensor(out=h[:], in0=hsb[:], in1=W["B1"][:], op=mybir.AluOpType.add)
                m = poolB.tile([128, 256], dt.float32, tag="mb")
                nc.vector.tensor_scalar_min(m[:], h[:], 0.0)
                nc.scalar.activation(m[:], m[:], mybir.ActivationFunctionType.Exp)
                r = poolB.tile([128, 256], dt.float32, tag="rb")
                nc.vector.tensor_scalar_max(r[:], h[:], 0.0)
                nc.vector.tensor_tensor(out=r[:], in0=r[:], in1=m[:], op=mybir.AluOpType.add)
                he = poolB.tile([128, 256], dt.bfloat16, tag="heb")
                nc.vector.tensor_scalar_add(he[:], r[:], -1.0)
                # transpose -> heT [2][128,128]
                heT = poolB.tile([128, 2, 128], dt.bfloat16, tag="heT")
                for c in range(2):
                    pt_ = psB.tile([128, 128], dt.bfloat16, tag="ptr")
                    nc.tensor.transpose(pt_[:], he[:, c * 128:(c + 1) * 128], W["IDENT"][:])
                    nc.scalar.copy(heT[:, c, :], pt_[:])
                ps2 = psB.tile([128, 258], dt.float32, tag="ps2")
                for c in range(2):
                    nc.tensor.matmul(ps2[:], heT[:, c, :], W["W2e"][:, c, :], start=(c == 0), stop=(c == 1))
                tab = poolB.tile([128, ROW], dt.bfloat16, tag="tab2")
                nc.scalar.copy(tab[:, 0:OUT], ps2[:, 0:OUT])
                nc.vector.memset(tab[:, OUT:OUT + 1], 1.0)
                nc.scalar.copy(tab[:, NRHS2:NRHS2 + 2], ps2[:, OUT:OUT + 2])
                nc.vector.memset(tab[:, NRHS2 + 2:ROW], 0.0)
                lo = b * 128
                nr = min(128, NSH - lo)
                if nr > 0:
                    nc.sync.dma_start(out=tab2_loc[lo:lo + nr], in_=tab[0:nr])

            if stop_after not in ('A', 'AG1'):
                edge_phase(tab1_ag, H, SLOT1, NRHS1, consume_l1)

        if stop_after not in ('A', 'AG1', 'B') and benchmode is None:
            nc.gpsimd.collective_compute(
                "AllGather", mybir.AluOpType.bypass,
                ins=[tab2_loc.opt()], outs=[tab2_ag.opt()], replica_groups=[core_ids])

        # ---------------- phase C: edge L2 + gate + h_final + A ----------------
        with tc.tile_pool(name="pc2", bufs=2) as poolC, \
             tc.tile_pool(name="psC", bufs=2, space="PSUM") as psC:

            def consume_l2(b, pool, hsb):
                hg = poolC.tile([128, 256], dt.float32, tag="hg")
                nc.vector.tensor_tensor(out=hg[:], in0=hsb[:], in1=W["B2"][:], op=mybir.AluOpType.add)
                hgb = poolC.tile([128, 256], dt.bfloat16, tag="hgb")
                nc.vector.tensor_copy(hgb[:], hg[:])
                giT = poolC.tile([128, 2, 128], dt.bfloat16, tag="giT")
                for c in range(2):
                    pt_ = psC.tile([128, 128], dt.bfloat16, tag="ptr2")
                    nc.tensor.transpose(pt_[:], hgb[:, c * 128:(c + 1) * 128], W["IDENT"][:])
                    nc.scalar.copy(giT[:, c, :], pt_[:])
                psg = psC.tile([128, 256], dt.float32, tag="psg")
                for c in range(2):
                    nc.tensor.matmul(psg[:], xT[:, c, b * 128:(b + 1) * 128], W["GW1"][:, c, :],
                                     start=(c == 0), stop=False)
                for c in range(2):
                    nc.tensor.matmul(psg[:], giT[:, c, :], W["GW1"][:, c + 2, :],
                                     start=False, stop=(c == 1))
                z1 = poolC.tile([128, 256], dt.float32, tag="z1")
                nc.vector.tensor_tensor(out=z1[:], in0=psg[:], in1=W["GB1"][:], op=mybir.AluOpType.add)
                nc.scalar.activation(z1[:], z1[:], mybir.ActivationFunctionType.Relu)
                zp = poolC.tile([128, 256], dt.float32, tag="zp")
                nc.vector.tensor_tensor(out=zp[:], in0=z1[:], in1=W["GW2"][:], op=mybir.AluOpType.mult)
                g = poolC.tile([128, 1], dt.float32, tag="g")
                nc.vector.tensor_reduce(g[:], zp[:], mybir.AxisListType.X, mybir.AluOpType.add)
                nc.scalar.activation(g[:], g[:], mybir.ActivationFunctionType.Sigmoid,
                                     bias=W["GB2"][:, 0:1])
                gm = poolC.tile([128, 1], dt.float32, tag="gm")
                nc.vector.tensor_scalar(gm[:], g[:], -1.0, 1.0,
                                        mybir.AluOpType.mult, mybir.AluOpType.add)
                xb = poolC.tile([128, 256], dt.float32, tag="xb")
                nc.sync.dma_start(out=xb[:], in_=x_ext[b * 128:(b + 1) * 128])
                t1 = poolC.tile([128, 256], dt.float32, tag="t1")
                nc.vector.tensor_scalar(t1[:], xb[:], gm[:, 0:1], None, mybir.AluOpType.mult)
                t2 = poolC.tile([128, 256], dt.float32, tag="t2")
                nc.vector.tensor_scalar(t2[:], hg[:], g[:, 0:1], None, mybir.AluOpType.mult)
                hf = poolC.tile([128, 256], dt.bfloat16, tag="hf")
                nc.vector.tensor_tensor(out=hf[:], in0=t1[:], in1=t2[:], op=mybir.AluOpType.add)
                lo = b * 128
                nr = min(128, NSH - lo)
                if nr > 0:
                    nc.sync.dma_start(out=hf_loc[lo:lo + nr], in_=hf[0:nr])
                # A = hf @ pw1_top + pb1
                hfT = poolC.tile([128, 2, 128], dt.bfloat16, tag="hfT")
                for c in range(2):
                    pt_ = psC.tile([128, 128], dt.bfloat16, tag="ptr2")
                    nc.tensor.transpose(pt_[:], hf[:, c * 128:(c + 1) * 128], W["IDENT"][:])
                    nc.scalar.copy(hfT[:, c, :], pt_[:])
                psa = psC.tile([128, PH], dt.float32, tag="psa")
                for c in range(2):
                    nc.tensor.matmul(psa[:], hfT[:, c, :], W["PWT"][:, c, :], start=(c == 0), stop=(c == 1))
                ab = poolC.tile([128, PH], dt.bfloat16, tag="ab")
                nc.vector.tensor_tensor(out=ab[:], in0=psa[:], in1=W["PB1"][:], op=mybir.AluOpType.add)
                nc.sync.dma_start(out=a_loc[b * 128:(b + 1) * 128], in_=ab[:])

            if stop_after not in ('A', 'AG1', 'B', 'AG2'):
                edge_phase(tab2_ag, 1, NRHS2, NRHS2, consume_l2)

        if stop_after not in ('A', 'AG1', 'B', 'AG2', 'C') and benchmode is None:
            nc.gpsimd.collective_compute(
                "AllGather", mybir.AluOpType.bypass,
                ins=[hf_loc.opt()], outs=[hf_ag.opt()], replica_groups=[core_ids])

        # ---------------- phase E: pairs ----------------
        hfv = hf_ag[:, :, :].flatten_outer_dims()   # [20000, 256]
        av = a_loc[:, :]                            # [2560, 512]
        with tc.tile_pool(name="pp", bufs=3) as pool, \
             tc.tile_pool(name="psZ", bufs=1, space="PSUM") as psZ, \
             tc.tile_pool(name="psO", bufs=2, space="PSUM") as psO:
            for j in range(nbat if stop_after == 'full' else 0):
                ui = pool.tile([128, PBATCH // 16], dt.int16, tag="ui")
                nc.sync.dma_start(out=ui[:], in_=uidx_ext[j])
                vi = pool.tile([128, PBATCH // 16], dt.int16, tag="vi")
                nc.sync.dma_start(out=vi[:], in_=vidx_ext[j])
                ug = pool.tile([128, 4, PBATCH], dt.bfloat16, tag="ug")
                nc.gpsimd.dma_gather(ug[:], av, ui[:], PBATCH, PBATCH, PH, transpose=True)
                vg = pool.tile([128, 2, PBATCH], dt.bfloat16, tag="vg")
                nc.gpsimd.dma_gather(vg[:], hfv, vi[:], PBATCH, PBATCH, OUT, transpose=True)
                psz = psZ.tile([128, 4, PBATCH], dt.float32, tag="psz")
                for m in range(4):
                    for c in range(2):
                        nc.tensor.matmul(psz[:, m, :], W["PWB"][:, c, m * 128:(m + 1) * 128],
                                         vg[:, c, :], start=(c == 0), stop=(c == 1))
                zr = pool.tile([128, 4, PBATCH], dt.bfloat16, tag="zr")
                nc.vector.tensor_tensor(out=zr[:], in0=psz[:], in1=ug[:], op=mybir.AluOpType.add)
                nc.scalar.activation(zr[:], zr[:], mybir.ActivationFunctionType.Relu)
                pso = psO.tile([1, PBATCH], dt.float32, tag="pso")
                for c in range(4):
                    nc.tensor.matmul(pso[:], W["PW2T"][:, c:c + 1], zr[:, c, :],
                                     start=(c == 0), stop=(c == 3))
                ob = pool.tile([1, PBATCH], dt.float32, tag="ob")
                nc.vector.tensor_scalar(ob[:], pso[:], W["PB2"][0:1, 0:1], None, mybir.AluOpType.add)
                nc.sync.dma_start(out=out_ext[j:j + 1, :], in_=ob[:])

        _benchstack.close()
        cpool.release()
        dram.release()
    nc.finalize()
    return nc


def build_coll(K):
    """Collective-only program: K repetitions of the kernel's 3 AllGathers."""
    nc = bacc.Bacc("TRN2", target_bir_lowering=False, debug=False, num_devices=NCORES)
    core_ids = list(range(NCORES))
    dummy_in = nc.declare_dram_parameter("dummy", [1, 16], dt.float32, isOutput=False)
    dummy_out = nc.declare_dram_parameter("dout", [1, 16], dt.float32, isOutput=True)
    with tile.TileContext(nc) as tc:
        dram = tc.alloc_tile_pool(name="dram", bufs=1, space="DRAM")
        tab1_loc = dram.tile([NSH, ROW], dt.bfloat16)
        tab1_ag = dram.tile([NCORES, NSH, ROW], dt.bfloat16, addr_space="Shared")
        tab2_loc = dram.tile([NSH, ROW], dt.bfloat16)
        tab2_ag = dram.tile([NCORES, NSH, ROW], dt.bfloat16, addr_space="Shared")
        hf_loc = dram.tile([NSH, OUT], dt.bfloat16)
        hf_ag = dram.tile([NCORES, NSH, OUT], dt.bfloat16, addr_space="Shared")
        for _ in range(K):
            nc.gpsimd.collective_compute(
                "AllGather", mybir.AluOpType.bypass,
                ins=[tab1_loc.opt()], outs=[tab1_ag.opt()], replica_groups=[core_ids])
            nc.gpsimd.collective_compute(
                "AllGather", mybir.AluOpType.bypass,
                ins=[tab2_loc.opt()], outs=[tab2_ag.opt()], replica_groups=[core_ids])
            nc.gpsimd.collective_compute(
                "AllGather", mybir.AluOpType.bypass,
                ins=[hf_loc.opt()], outs=[hf_ag.opt()], replica_groups=[core_ids])
        spool = tc.alloc_tile_pool(name="s", bufs=1)
        t = spool.tile([1, 16], dt.float32)
        nc.sync.dma_start(out=t[:], in_=dummy_in[:])
        nc.sync.dma_start(out=dummy_out[:], in_=t[:])
        spool.release()
        dram.release()
    nc.finalize()
    return nc


def kernel(**inputs):
    import time
    edge_index = np.asarray(inputs["edge_index"])
    u_nodes = np.asarray(inputs["u_nodes"])
    v_nodes = np.asarray(inputs["v_nodes"])
    x = np.asarray(inputs["x"], FP32)

    t0 = time.time()
    cores, pmax, nbat = preprocess(edge_index, u_nodes, v_nodes)
    w = host_weights(inputs)
    in_maps = []
    for k in range(NCORES):
        xs = np.zeros((NSH_PAD, 256), FP32)
        xs[:NSH] = x[k * NSH:(k + 1) * NSH]
        xT = np.ascontiguousarray(
            xs.T.reshape(2, 128, NSH_PAD).transpose(1, 0, 2)).astype(BF16)
        c = cores[k]
        m = dict(xT=xT, x=xs, esrc=c["esrc"], edst=c["edst"], erel=c["erel"],
                 uidx=c["uidx"], vidx=c["vidx"])
        m.update(w)
        in_maps.append(m)
    t1 = time.time()
    nc = build(nbat, pmax)
    t2 = time.time()
    res = run_bass_kernel_spmd(nc, in_maps, list(range(NCORES)))
    t3 = time.time()
    out_full = np.zeros(P, FP32)
    for k in range(NCORES):
        o = res.results[k]["pout"].reshape(-1)
        out_full[cores[k]["pidx"]] = o[:cores[k]["np_real"]]
    print(f"[gfk] prep {t1-t0:.2f}s build {t2-t1:.2f}s run {t3-t2:.2f}s", file=sys.stderr)
    return out_full



